# revision 3
# baseline (speedup 1.0000x reference)
"""BlockSparseLinearWithPerm Trainium2 kernel.

Math: out[b,s,j] = sum_i x[b,s,in_perm[i]] * W[out_perm[j], i] + bias[out_perm[j]]
where W is the dense form of the block-sparse weight.

Both permutations and the block scatter are folded on the host into a dense
effective weight  W_effT[k, j] = sum_{i: in_perm[i]==k} W[out_perm[j], i]
(host cost: one 1024x1024 scatter-add — negligible), so the device kernel is a
pure dense matmul  out = x @ W_effT + bias_eff, data-parallel over the batch
dim: one batch element (8192x1024 tokens) per NeuronCore, weights replicated.

Device pipeline per 128-token tile (all matmul dtypes are float32r — full
1 cycle/row PE rate, tf32-class mantissa, ~1e-3 scale-relative error):
  DMA x tile [128, 1024] -> PE transpose k-tiles (via identity) -> PSUM
  -> ScalarE copy x^T to SBUF -> 16 accumulating f32r matmuls against the
  resident W_effT -> VectorE adds bias while copying PSUM -> SBUF -> DMA out.
"""
import os
import sys
import subprocess
import tempfile

import numpy as np

_TRN_REPO = "/opt/trn_rl_repo"

D_IN = 1024
D_OUT = 1024
BS = 64
R = D_OUT // BS
C = D_IN // BS
BATCH = 8
SEQ = 8192
P = 128
KT = D_IN // P          # 8 k-tiles
ST = SEQ // P           # 64 token tiles per core
N_CORES = 8


def _fold_weights(weight_blocks, brow, bcol, bias, in_perm, out_perm):
    """Fold block scatter + both permutations into W_effT [k, j] and bias_eff."""
    wb = np.asarray(weight_blocks, dtype=np.float64)
    brow = np.asarray(brow).astype(np.int64)
    bcol = np.asarray(bcol).astype(np.int64)
    in_perm = np.asarray(in_perm).astype(np.int64)
    out_perm = np.asarray(out_perm).astype(np.int64)
    W4 = np.zeros((R, C, BS, BS), dtype=np.float64)
    W4[brow, bcol] = wb
    W = W4.transpose(0, 2, 1, 3).reshape(D_OUT, D_IN)
    Wp = W[out_perm]                       # [j, i]
    W_effT = np.zeros((D_IN, D_OUT), dtype=np.float64)
    np.add.at(W_effT, in_perm, Wp.T)       # row i of Wp.T added into row in_perm[i]
    bias_eff = np.asarray(bias, dtype=np.float64)[out_perm]
    bias_bcast = np.broadcast_to(bias_eff, (P, D_OUT)).copy()
    return (np.ascontiguousarray(W_effT, dtype=np.float64).astype(np.float32),
            bias_bcast.astype(np.float32))


_NC_CACHE = {}


def _build_nc():
    if "nc" in _NC_CACHE:
        return _NC_CACHE["nc"]
    if _TRN_REPO not in sys.path:
        sys.path.insert(0, _TRN_REPO)
    import concourse.bacc as bacc
    import concourse.mybir as mybir
    from concourse.tile import TileContext
    from concourse.masks import make_identity
    from contextlib import ExitStack

    F32 = mybir.dt.float32
    F32R = mybir.dt.float32r

    nc = bacc.Bacc(target_bir_lowering=False)
    x_d = nc.declare_dram_parameter("x", [SEQ, D_IN], F32R, isOutput=False)
    wt_d = nc.declare_dram_parameter("wt", [D_IN, D_OUT], F32R, isOutput=False)
    bias_d = nc.declare_dram_parameter("bias", [P, D_OUT], F32, isOutput=False)
    out_d = nc.declare_dram_parameter("out", [SEQ, D_OUT], F32, isOutput=True)

    with TileContext(nc) as tc, ExitStack() as ctx:
        consts = ctx.enter_context(tc.tile_pool(name="consts", bufs=1))
        xpool = ctx.enter_context(tc.tile_pool(name="xpool", bufs=3))
        xtpool = ctx.enter_context(tc.tile_pool(name="xtpool", bufs=2))
        opool = ctx.enter_context(tc.tile_pool(name="opool", bufs=3))
        ps_xt = ctx.enter_context(tc.tile_pool(name="ps_xt", bufs=2, space="PSUM"))
        ps_o = ctx.enter_context(tc.tile_pool(name="ps_o", bufs=2, space="PSUM"))

        ident = consts.tile([P, P], F32)
        make_identity(nc, ident)
        ident_r = consts.tile([P, P], F32R)
        nc.vector.tensor_copy(out=ident_r, in_=ident)

        w_sb = consts.tile([P, KT, D_OUT], F32R)
        nc.sync.dma_start(out=w_sb, in_=wt_d.rearrange("(kt p) j -> p kt j", p=P))
        bias_sb = consts.tile([P, D_OUT], F32)
        nc.sync.dma_start(out=bias_sb, in_=bias_d[:, :])

        for st in range(ST):
            x_sb = xpool.tile([P, D_IN], F32R, tag="x")
            nc.sync.dma_start(out=x_sb, in_=x_d[st * P:(st + 1) * P, :])

            xt_halves = []
            for h in range(2):
                ptx = ps_xt.tile([P, 4, P], F32R, tag=f"ptx{h}")
                for i in range(4):
                    kt = h * 4 + i
                    nc.tensor.transpose(
                        ptx[:, i], x_sb[:, kt * P:(kt + 1) * P], ident_r)
                xt_sb = xtpool.tile([P, 4, P], F32R, tag=f"xt{h}")
                nc.scalar.copy(out=xt_sb, in_=ptx)
                xt_halves.append(xt_sb)

            out_sb = opool.tile([P, D_OUT], F32, tag="o")
            pos = [ps_o.tile([P, 512], F32, tag=f"po{jh}", name=f"po{jh}") for jh in range(2)]
            for kt in range(KT):
                lhsT = xt_halves[kt // 4][:, kt % 4]
                for jh in range(2):
                    nc.tensor.matmul(
                        pos[jh], lhsT, w_sb[:, kt, jh * 512:(jh + 1) * 512],
                        start=(kt == 0), stop=(kt == KT - 1))
            for jh in range(2):
                nc.vector.tensor_add(
                    out=out_sb[:, jh * 512:(jh + 1) * 512],
                    in0=pos[jh],
                    in1=bias_sb[:, jh * 512:(jh + 1) * 512])
            nc.sync.dma_start(out=out_d[st * P:(st + 1) * P, :], in_=out_sb)

    nc.finalize()
    _NC_CACHE["nc"] = nc
    return nc


def _run_device(x, W_effT, bias_bcast, trace=False, tmpdir=None):
    """Run the SPMD kernel on 8 cores in this process. Returns (out, exec_ns)."""
    if _TRN_REPO not in sys.path:
        sys.path.insert(0, _TRN_REPO)
    from concourse.bass_utils import run_bass_kernel_spmd

    nc = _build_nc()
    core_ids = list(range(N_CORES))
    in_maps = [
        {"x": np.ascontiguousarray(x[c], dtype=np.float32),
         "wt": W_effT, "bias": bias_bcast}
        for c in core_ids
    ]
    res = run_bass_kernel_spmd(nc, in_maps, core_ids, trace=trace, tmpdir=tmpdir)
    out = np.stack([res.results[c]["out"] for c in core_ids], axis=0)
    return out, res.exec_time_ns


def _kernel_impl(x, in_perm, out_perm, weight_blocks, brow, bcol, bias, trace=False, tmpdir=None):
    x = np.asarray(x)
    W_effT, bias_bcast = _fold_weights(
        weight_blocks, brow, bcol, bias, in_perm, out_perm)
    out, exec_ns = _run_device(
        x.reshape(BATCH, SEQ, D_IN), W_effT, bias_bcast, trace=trace, tmpdir=tmpdir)
    return out.astype(np.float32), exec_ns


def kernel(x, in_perm, out_perm, weight_blocks, brow, bcol, bias):
    try:
        out, _ = _kernel_impl(
            x, in_perm, out_perm, weight_blocks, brow, bcol, bias)
        return out
    except Exception:
        # Fallback: run the device part in a clean subprocess (e.g. if the
        # calling process pinned JAX_PLATFORMS=cpu before importing jax).
        return _kernel_subprocess(
            x, in_perm, out_perm, weight_blocks, brow, bcol, bias)


def _kernel_subprocess(x, in_perm, out_perm, weight_blocks, brow, bcol, bias):
    with tempfile.TemporaryDirectory() as td:
        inp = os.path.join(td, "in.npz")
        outp = os.path.join(td, "out.npy")
        np.savez(inp, x=x, in_perm=in_perm, out_perm=out_perm,
                 weight_blocks=weight_blocks, brow=brow, bcol=bcol, bias=bias)
        env = dict(os.environ)
        env.pop("JAX_PLATFORMS", None)
        subprocess.run(
            [sys.executable, os.path.abspath(__file__), "--serve", inp, outp],
            check=True, env=env)
        return np.load(outp)


if __name__ == "__main__":
    if len(sys.argv) == 4 and sys.argv[1] == "--serve":
        data = np.load(sys.argv[2])
        out, _ = _kernel_impl(
            data["x"], data["in_perm"], data["out_perm"],
            data["weight_blocks"], data["brow"], data["bcol"], data["bias"])
        np.save(sys.argv[3], out)


# revision 4
# speedup vs baseline: 1.0009x; 1.0009x over previous
"""BlockSparseLinearWithPerm Trainium2 kernel.

Math: out[b,s,j] = sum_i x[b,s,in_perm[i]] * W[out_perm[j], i] + bias[out_perm[j]]
where W is the dense form of the block-sparse weight.

Both permutations and the block scatter are folded on the host into a dense
effective weight  W_effT[k, j] = sum_{i: in_perm[i]==k} W[out_perm[j], i]
(host cost: one 1024x1024 scatter-add — negligible), so the device kernel is a
pure dense matmul  out = x @ W_effT + bias_eff, data-parallel over the batch
dim: one batch element (8192x1024 tokens) per NeuronCore, weights replicated.

Device pipeline per 128-token tile (all matmul dtypes are float32r — full
1 cycle/row PE rate, tf32-class mantissa, ~1e-3 scale-relative error):
  DMA x tile [128, 1024] -> PE transpose k-tiles (via identity) -> PSUM
  -> ScalarE copy x^T to SBUF -> 16 accumulating f32r matmuls against the
  resident W_effT -> VectorE adds bias while copying PSUM -> SBUF -> DMA out.
"""
import os
import sys
import subprocess
import tempfile

import numpy as np

_TRN_REPO = "/opt/trn_rl_repo"

D_IN = 1024
D_OUT = 1024
BS = 64
R = D_OUT // BS
C = D_IN // BS
BATCH = 8
SEQ = 8192
P = 128
KT = D_IN // P          # 8 k-tiles
ST = SEQ // P           # 64 token tiles per core
N_CORES = 8


def _fold_weights(weight_blocks, brow, bcol, bias, in_perm, out_perm):
    """Fold block scatter + both permutations into W_effT [k, j] and bias_eff."""
    wb = np.asarray(weight_blocks, dtype=np.float64)
    brow = np.asarray(brow).astype(np.int64)
    bcol = np.asarray(bcol).astype(np.int64)
    in_perm = np.asarray(in_perm).astype(np.int64)
    out_perm = np.asarray(out_perm).astype(np.int64)
    W4 = np.zeros((R, C, BS, BS), dtype=np.float64)
    W4[brow, bcol] = wb
    W = W4.transpose(0, 2, 1, 3).reshape(D_OUT, D_IN)
    Wp = W[out_perm]                       # [j, i]
    W_effT = np.zeros((D_IN, D_OUT), dtype=np.float64)
    np.add.at(W_effT, in_perm, Wp.T)       # row i of Wp.T added into row in_perm[i]
    bias_eff = np.asarray(bias, dtype=np.float64)[out_perm]
    bias_bcast = np.broadcast_to(bias_eff, (P, D_OUT)).copy()
    return (np.ascontiguousarray(W_effT, dtype=np.float64).astype(np.float32),
            bias_bcast.astype(np.float32))


_NC_CACHE = {}


def _build_nc():
    if "nc" in _NC_CACHE:
        return _NC_CACHE["nc"]
    if _TRN_REPO not in sys.path:
        sys.path.insert(0, _TRN_REPO)
    import concourse.bacc as bacc
    import concourse.mybir as mybir
    from concourse.tile import TileContext
    from concourse.masks import make_identity
    from contextlib import ExitStack

    F32 = mybir.dt.float32
    F32R = mybir.dt.float32r

    nc = bacc.Bacc(target_bir_lowering=False)
    x_d = nc.declare_dram_parameter("x", [SEQ, D_IN], F32R, isOutput=False)
    wt_d = nc.declare_dram_parameter("wt", [D_IN, D_OUT], F32R, isOutput=False)
    bias_d = nc.declare_dram_parameter("bias", [P, D_OUT], F32, isOutput=False)
    out_d = nc.declare_dram_parameter("out", [SEQ, D_OUT], F32, isOutput=True)

    with TileContext(nc) as tc, ExitStack() as ctx:
        consts = ctx.enter_context(tc.tile_pool(name="consts", bufs=1))
        xpool = ctx.enter_context(tc.tile_pool(name="xpool", bufs=4))
        xtpool = ctx.enter_context(tc.tile_pool(name="xtpool", bufs=2))
        opool = ctx.enter_context(tc.tile_pool(name="opool", bufs=4))
        ps_xt = ctx.enter_context(tc.tile_pool(name="ps_xt", bufs=2, space="PSUM"))
        ps_o = ctx.enter_context(tc.tile_pool(name="ps_o", bufs=2, space="PSUM"))

        ident = consts.tile([P, P], F32)
        make_identity(nc, ident)
        ident_r = consts.tile([P, P], F32R)
        nc.vector.tensor_copy(out=ident_r, in_=ident)

        # x loads ride the SP HWDGE ring; W/bias ride the ACT HWDGE ring so the
        # 4.5MB of constants don't FIFO-block the first x tiles; out stores go
        # through SWDGE (gpsimd).  W is chunked per k-tile so the first matmuls
        # only wait on the first 512KB.
        w_sb = consts.tile([P, KT, D_OUT], F32R)
        wt_r = wt_d.rearrange("(kt p) j -> p kt j", p=P)
        bias_sb = consts.tile([P, D_OUT], F32)

        first_x = []
        for st in range(2):
            x_sb = xpool.tile([P, D_IN], F32R, tag="x", name="x_sb")
            nc.sync.dma_start(out=x_sb, in_=x_d[st * P:(st + 1) * P, :])
            first_x.append(x_sb)
        for kt in range(KT):
            nc.scalar.dma_start(out=w_sb[:, kt], in_=wt_r[:, kt])
        nc.scalar.dma_start(out=bias_sb, in_=bias_d[:, :])

        for st in range(ST):
            if st < 2:
                x_sb = first_x[st]
            else:
                x_sb = xpool.tile([P, D_IN], F32R, tag="x", name="x_sb")
                nc.sync.dma_start(out=x_sb, in_=x_d[st * P:(st + 1) * P, :])

            xt_halves = []
            for h in range(2):
                ptx = ps_xt.tile([P, 4, P], F32R, tag=f"ptx{h}")
                for i in range(4):
                    kt = h * 4 + i
                    nc.tensor.transpose(
                        ptx[:, i], x_sb[:, kt * P:(kt + 1) * P], ident_r)
                xt_sb = xtpool.tile([P, 4, P], F32R, tag=f"xt{h}")
                nc.scalar.copy(out=xt_sb, in_=ptx)
                xt_halves.append(xt_sb)

            out_sb = opool.tile([P, D_OUT], F32, tag="o")
            pos = [ps_o.tile([P, 512], F32, tag=f"po{jh}", name=f"po{jh}") for jh in range(2)]
            for kt in range(KT):
                lhsT = xt_halves[kt // 4][:, kt % 4]
                for jh in range(2):
                    nc.tensor.matmul(
                        pos[jh], lhsT, w_sb[:, kt, jh * 512:(jh + 1) * 512],
                        start=(kt == 0), stop=(kt == KT - 1))
            for jh in range(2):
                nc.vector.tensor_add(
                    out=out_sb[:, jh * 512:(jh + 1) * 512],
                    in0=pos[jh],
                    in1=bias_sb[:, jh * 512:(jh + 1) * 512])
            nc.gpsimd.dma_start(out=out_d[st * P:(st + 1) * P, :], in_=out_sb)

    nc.finalize()
    _NC_CACHE["nc"] = nc
    return nc


def _run_device(x, W_effT, bias_bcast, trace=False, tmpdir=None):
    """Run the SPMD kernel on 8 cores in this process. Returns (out, exec_ns)."""
    if _TRN_REPO not in sys.path:
        sys.path.insert(0, _TRN_REPO)
    from concourse.bass_utils import run_bass_kernel_spmd

    nc = _build_nc()
    core_ids = list(range(N_CORES))
    in_maps = [
        {"x": np.ascontiguousarray(x[c], dtype=np.float32),
         "wt": W_effT, "bias": bias_bcast}
        for c in core_ids
    ]
    res = run_bass_kernel_spmd(nc, in_maps, core_ids, trace=trace, tmpdir=tmpdir)
    out = np.stack([res.results[c]["out"] for c in core_ids], axis=0)
    return out, res.exec_time_ns


def _kernel_impl(x, in_perm, out_perm, weight_blocks, brow, bcol, bias, trace=False, tmpdir=None):
    x = np.asarray(x)
    W_effT, bias_bcast = _fold_weights(
        weight_blocks, brow, bcol, bias, in_perm, out_perm)
    out, exec_ns = _run_device(
        x.reshape(BATCH, SEQ, D_IN), W_effT, bias_bcast, trace=trace, tmpdir=tmpdir)
    return out.astype(np.float32), exec_ns


def kernel(x, in_perm, out_perm, weight_blocks, brow, bcol, bias):
    try:
        out, _ = _kernel_impl(
            x, in_perm, out_perm, weight_blocks, brow, bcol, bias)
        return out
    except Exception:
        # Fallback: run the device part in a clean subprocess (e.g. if the
        # calling process pinned JAX_PLATFORMS=cpu before importing jax).
        return _kernel_subprocess(
            x, in_perm, out_perm, weight_blocks, brow, bcol, bias)


def _kernel_subprocess(x, in_perm, out_perm, weight_blocks, brow, bcol, bias):
    with tempfile.TemporaryDirectory() as td:
        inp = os.path.join(td, "in.npz")
        outp = os.path.join(td, "out.npy")
        np.savez(inp, x=x, in_perm=in_perm, out_perm=out_perm,
                 weight_blocks=weight_blocks, brow=brow, bcol=bcol, bias=bias)
        env = dict(os.environ)
        env.pop("JAX_PLATFORMS", None)
        subprocess.run(
            [sys.executable, os.path.abspath(__file__), "--serve", inp, outp],
            check=True, env=env)
        return np.load(outp)


if __name__ == "__main__":
    if len(sys.argv) == 4 and sys.argv[1] == "--serve":
        data = np.load(sys.argv[2])
        out, _ = _kernel_impl(
            data["x"], data["in_perm"], data["out_perm"],
            data["weight_blocks"], data["brow"], data["bcol"], data["bias"])
        np.save(sys.argv[3], out)


# revision 5
# speedup vs baseline: 1.0014x; 1.0004x over previous
"""BlockSparseLinearWithPerm Trainium2 kernel.

Math: out[b,s,j] = sum_i x[b,s,in_perm[i]] * W[out_perm[j], i] + bias[out_perm[j]]
where W is the dense form of the block-sparse weight.

Both permutations and the block scatter are folded on the host into a dense
effective weight  W_effT[k, j] = sum_{i: in_perm[i]==k} W[out_perm[j], i]
(host cost: one 1024x1024 scatter-add — negligible), so the device kernel is a
pure dense matmul  out = x @ W_effT + bias_eff, data-parallel over the batch
dim: one batch element (8192x1024 tokens) per NeuronCore, weights replicated.

Device pipeline per 128-token tile (all matmul dtypes are float32r — full
1 cycle/row PE rate, tf32-class mantissa, ~1e-3 scale-relative error):
  DMA x tile [128, 1024] -> PE transpose k-tiles (via identity) -> PSUM
  -> ScalarE copy x^T to SBUF -> 16 accumulating f32r matmuls against the
  resident W_effT -> VectorE adds bias while copying PSUM -> SBUF -> DMA out.
"""
import os
import sys
import subprocess
import tempfile

import numpy as np

_TRN_REPO = "/opt/trn_rl_repo"

D_IN = 1024
D_OUT = 1024
BS = 64
R = D_OUT // BS
C = D_IN // BS
BATCH = 8
SEQ = 8192
P = 128
KT = D_IN // P          # 8 k-tiles
ST = SEQ // P           # 64 token tiles per core
N_CORES = 8


def _fold_weights(weight_blocks, brow, bcol, bias, in_perm, out_perm):
    """Fold block scatter + both permutations into W_effT [k, j] and bias_eff."""
    wb = np.asarray(weight_blocks, dtype=np.float64)
    brow = np.asarray(brow).astype(np.int64)
    bcol = np.asarray(bcol).astype(np.int64)
    in_perm = np.asarray(in_perm).astype(np.int64)
    out_perm = np.asarray(out_perm).astype(np.int64)
    W4 = np.zeros((R, C, BS, BS), dtype=np.float64)
    W4[brow, bcol] = wb
    W = W4.transpose(0, 2, 1, 3).reshape(D_OUT, D_IN)
    Wp = W[out_perm]                       # [j, i]
    W_effT = np.zeros((D_IN, D_OUT), dtype=np.float64)
    np.add.at(W_effT, in_perm, Wp.T)       # row i of Wp.T added into row in_perm[i]
    bias_eff = np.asarray(bias, dtype=np.float64)[out_perm]
    bias_bcast = np.broadcast_to(bias_eff, (P, D_OUT)).copy()
    return (np.ascontiguousarray(W_effT, dtype=np.float64).astype(np.float32),
            bias_bcast.astype(np.float32))


_NC_CACHE = {}


def _build_nc():
    if "nc" in _NC_CACHE:
        return _NC_CACHE["nc"]
    if _TRN_REPO not in sys.path:
        sys.path.insert(0, _TRN_REPO)
    import concourse.bacc as bacc
    import concourse.mybir as mybir
    from concourse.tile import TileContext
    from concourse.masks import make_identity
    from contextlib import ExitStack

    F32 = mybir.dt.float32
    F32R = mybir.dt.float32r

    nc = bacc.Bacc(target_bir_lowering=False)
    x_d = nc.declare_dram_parameter("x", [SEQ, D_IN], F32R, isOutput=False)
    wt_d = nc.declare_dram_parameter("wt", [D_IN, D_OUT], F32R, isOutput=False)
    bias_d = nc.declare_dram_parameter("bias", [P, D_OUT], F32, isOutput=False)
    out_d = nc.declare_dram_parameter("out", [SEQ, D_OUT], F32, isOutput=True)

    with TileContext(nc) as tc, ExitStack() as ctx:
        consts = ctx.enter_context(tc.tile_pool(name="consts", bufs=1))
        xpool = ctx.enter_context(tc.tile_pool(name="xpool", bufs=4))
        xtpool = ctx.enter_context(tc.tile_pool(name="xtpool", bufs=2))
        opool = ctx.enter_context(tc.tile_pool(name="opool", bufs=4))
        ps_xt = ctx.enter_context(tc.tile_pool(name="ps_xt", bufs=2, space="PSUM"))
        ps_o = ctx.enter_context(tc.tile_pool(name="ps_o", bufs=2, space="PSUM"))

        ident = consts.tile([P, P], F32)
        make_identity(nc, ident)
        ident_r = consts.tile([P, P], F32R)
        nc.vector.tensor_copy(out=ident_r, in_=ident)

        # x loads ride the SP HWDGE ring; W/bias ride the ACT HWDGE ring so the
        # 4.5MB of constants don't FIFO-block the first x tiles; out stores go
        # through SWDGE (gpsimd).  W is chunked per k-tile so the first matmuls
        # only wait on the first 512KB.
        wt_r = wt_d.rearrange("(kt p) j -> p kt j", p=P)
        bias_sb = consts.tile([P, D_OUT], F32)

        first_x = []
        for st in range(2):
            x_sb = xpool.tile([P, D_IN], F32R, tag="x", name="x_sb")
            nc.sync.dma_start(out=x_sb, in_=x_d[st * P:(st + 1) * P, :])
            first_x.append(x_sb)
        w_tiles = []
        for kt in range(KT):
            w_kt = consts.tile([P, D_OUT], F32R, name=f"w_{kt}")
            nc.scalar.dma_start(out=w_kt, in_=wt_r[:, kt])
            w_tiles.append(w_kt)
        nc.scalar.dma_start(out=bias_sb, in_=bias_d[:, :])

        for st in range(ST):
            if st < 2:
                x_sb = first_x[st]
            else:
                x_sb = xpool.tile([P, D_IN], F32R, tag="x", name="x_sb")
                nc.sync.dma_start(out=x_sb, in_=x_d[st * P:(st + 1) * P, :])

            xt_halves = []
            for h in range(2):
                ptx = ps_xt.tile([P, 4, P], F32R, tag=f"ptx{h}")
                for i in range(4):
                    kt = h * 4 + i
                    nc.tensor.transpose(
                        ptx[:, i], x_sb[:, kt * P:(kt + 1) * P], ident_r)
                xt_sb = xtpool.tile([P, 4, P], F32R, tag=f"xt{h}")
                nc.scalar.copy(out=xt_sb, in_=ptx)
                xt_halves.append(xt_sb)

            out_sb = opool.tile([P, D_OUT], F32, tag="o")
            pos = [ps_o.tile([P, 512], F32, tag=f"po{jh}", name=f"po{jh}") for jh in range(2)]
            for kt in range(KT):
                lhsT = xt_halves[kt // 4][:, kt % 4]
                for jh in range(2):
                    nc.tensor.matmul(
                        pos[jh], lhsT, w_tiles[kt][:, jh * 512:(jh + 1) * 512],
                        start=(kt == 0), stop=(kt == KT - 1))
            for jh in range(2):
                nc.vector.tensor_add(
                    out=out_sb[:, jh * 512:(jh + 1) * 512],
                    in0=pos[jh],
                    in1=bias_sb[:, jh * 512:(jh + 1) * 512])
            nc.gpsimd.dma_start(out=out_d[st * P:(st + 1) * P, :], in_=out_sb)

    nc.finalize()
    _NC_CACHE["nc"] = nc
    return nc


def _run_device(x, W_effT, bias_bcast, trace=False, tmpdir=None):
    """Run the SPMD kernel on 8 cores in this process. Returns (out, exec_ns)."""
    if _TRN_REPO not in sys.path:
        sys.path.insert(0, _TRN_REPO)
    from concourse.bass_utils import run_bass_kernel_spmd

    nc = _build_nc()
    core_ids = list(range(N_CORES))
    in_maps = [
        {"x": np.ascontiguousarray(x[c], dtype=np.float32),
         "wt": W_effT, "bias": bias_bcast}
        for c in core_ids
    ]
    res = run_bass_kernel_spmd(nc, in_maps, core_ids, trace=trace, tmpdir=tmpdir)
    out = np.stack([res.results[c]["out"] for c in core_ids], axis=0)
    return out, res.exec_time_ns


def _kernel_impl(x, in_perm, out_perm, weight_blocks, brow, bcol, bias, trace=False, tmpdir=None):
    x = np.asarray(x)
    W_effT, bias_bcast = _fold_weights(
        weight_blocks, brow, bcol, bias, in_perm, out_perm)
    out, exec_ns = _run_device(
        x.reshape(BATCH, SEQ, D_IN), W_effT, bias_bcast, trace=trace, tmpdir=tmpdir)
    return out.astype(np.float32), exec_ns


def kernel(x, in_perm, out_perm, weight_blocks, brow, bcol, bias):
    try:
        out, _ = _kernel_impl(
            x, in_perm, out_perm, weight_blocks, brow, bcol, bias)
        return out
    except Exception:
        # Fallback: run the device part in a clean subprocess (e.g. if the
        # calling process pinned JAX_PLATFORMS=cpu before importing jax).
        return _kernel_subprocess(
            x, in_perm, out_perm, weight_blocks, brow, bcol, bias)


def _kernel_subprocess(x, in_perm, out_perm, weight_blocks, brow, bcol, bias):
    with tempfile.TemporaryDirectory() as td:
        inp = os.path.join(td, "in.npz")
        outp = os.path.join(td, "out.npy")
        np.savez(inp, x=x, in_perm=in_perm, out_perm=out_perm,
                 weight_blocks=weight_blocks, brow=brow, bcol=bcol, bias=bias)
        env = dict(os.environ)
        env.pop("JAX_PLATFORMS", None)
        subprocess.run(
            [sys.executable, os.path.abspath(__file__), "--serve", inp, outp],
            check=True, env=env)
        return np.load(outp)


if __name__ == "__main__":
    if len(sys.argv) == 4 and sys.argv[1] == "--serve":
        data = np.load(sys.argv[2])
        out, _ = _kernel_impl(
            data["x"], data["in_perm"], data["out_perm"],
            data["weight_blocks"], data["brow"], data["bcol"], data["bias"])
        np.save(sys.argv[3], out)


# revision 6
# speedup vs baseline: 1.0137x; 1.0123x over previous
"""BlockSparseLinearWithPerm Trainium2 kernel.

Math: out[b,s,j] = sum_i x[b,s,in_perm[i]] * W[out_perm[j], i] + bias[out_perm[j]]
where W is the dense form of the block-sparse weight.

Both permutations and the block scatter are folded on the host into a dense
effective weight  W_effT[k, j] = sum_{i: in_perm[i]==k} W[out_perm[j], i]
(host cost: one 1024x1024 scatter-add — negligible), so the device kernel is a
pure dense matmul  out = x @ W_effT + bias_eff, data-parallel over the batch
dim: one batch element (8192x1024 tokens) per NeuronCore, weights replicated.

Device pipeline per 128-token tile (all matmul dtypes are float32r — full
1 cycle/row PE rate, tf32-class mantissa, ~1e-3 scale-relative error):
  DMA x tile [128, 1024] -> PE transpose k-tiles (via identity) -> PSUM
  -> ScalarE copy x^T to SBUF -> 16 accumulating f32r matmuls against the
  resident W_effT -> VectorE adds bias while copying PSUM -> SBUF -> DMA out.
"""
import os
import sys
import subprocess
import tempfile

import numpy as np

_TRN_REPO = "/opt/trn_rl_repo"

D_IN = 1024
D_OUT = 1024
BS = 64
R = D_OUT // BS
C = D_IN // BS
BATCH = 8
SEQ = 8192
P = 128
KT = D_IN // P          # 8 k-tiles
ST = SEQ // P           # 64 token tiles per core
N_CORES = 8


def _fold_weights(weight_blocks, brow, bcol, bias, in_perm, out_perm):
    """Fold block scatter + both permutations into W_effT [k, j] and bias_eff."""
    wb = np.asarray(weight_blocks, dtype=np.float64)
    brow = np.asarray(brow).astype(np.int64)
    bcol = np.asarray(bcol).astype(np.int64)
    in_perm = np.asarray(in_perm).astype(np.int64)
    out_perm = np.asarray(out_perm).astype(np.int64)
    W4 = np.zeros((R, C, BS, BS), dtype=np.float64)
    W4[brow, bcol] = wb
    W = W4.transpose(0, 2, 1, 3).reshape(D_OUT, D_IN)
    Wp = W[out_perm]                       # [j, i]
    W_effT = np.zeros((D_IN, D_OUT), dtype=np.float64)
    np.add.at(W_effT, in_perm, Wp.T)       # row i of Wp.T added into row in_perm[i]
    bias_eff = np.asarray(bias, dtype=np.float64)[out_perm]
    bias_bcast = np.broadcast_to(bias_eff, (P, D_OUT)).copy()
    return (np.ascontiguousarray(W_effT, dtype=np.float64).astype(np.float32),
            bias_bcast.astype(np.float32))


_NC_CACHE = {}


def _build_nc():
    if "nc" in _NC_CACHE:
        return _NC_CACHE["nc"]
    if _TRN_REPO not in sys.path:
        sys.path.insert(0, _TRN_REPO)
    import concourse.bacc as bacc
    import concourse.mybir as mybir
    from concourse.tile import TileContext
    from concourse.masks import make_identity
    from contextlib import ExitStack

    F32 = mybir.dt.float32
    F32R = mybir.dt.float32r

    nc = bacc.Bacc(target_bir_lowering=False)
    x_d = nc.declare_dram_parameter("x", [SEQ, D_IN], F32R, isOutput=False)
    wt_d = nc.declare_dram_parameter("wt", [D_IN, D_OUT], F32R, isOutput=False)
    bias_d = nc.declare_dram_parameter("bias", [P, D_OUT], F32, isOutput=False)
    out_d = nc.declare_dram_parameter("out", [SEQ, D_OUT], F32, isOutput=True)

    with TileContext(nc) as tc, ExitStack() as ctx:
        consts = ctx.enter_context(tc.tile_pool(name="consts", bufs=1))
        xpool = ctx.enter_context(tc.tile_pool(name="xpool", bufs=4))
        xtpool = ctx.enter_context(tc.tile_pool(name="xtpool", bufs=2))
        opool = ctx.enter_context(tc.tile_pool(name="opool", bufs=4))
        ps_xt = ctx.enter_context(tc.tile_pool(name="ps_xt", bufs=2, space="PSUM"))
        ps_o = ctx.enter_context(tc.tile_pool(name="ps_o", bufs=2, space="PSUM"))

        ident = consts.tile([P, P], F32)
        make_identity(nc, ident)
        ident_r = consts.tile([P, P], F32R)
        nc.vector.tensor_copy(out=ident_r, in_=ident)

        # x loads ride the SP HWDGE ring; W/bias ride the ACT HWDGE ring so the
        # 4.5MB of constants don't FIFO-block the first x tiles; out stores go
        # through SWDGE (gpsimd).  W is chunked per k-tile so the first matmuls
        # only wait on the first 512KB.
        wt_r = wt_d.rearrange("(kt p) j -> p kt j", p=P)
        bias_sb = consts.tile([P, D_OUT], F32)

        # W chunks 0-1 go FIRST on the sync ring so the first matmuls'
        # weights land before the x backlog; the rest stream on the scalar
        # ring in parallel with x loads.
        w_tiles = []
        for kt in range(KT):
            w_kt = consts.tile([P, D_OUT], F32R, name=f"w_{kt}")
            if kt < 2:
                nc.sync.dma_start(out=w_kt, in_=wt_r[:, kt])
            else:
                nc.scalar.dma_start(out=w_kt, in_=wt_r[:, kt])
            w_tiles.append(w_kt)
        nc.scalar.dma_start(out=bias_sb, in_=bias_d[:, :])
        first_x = []
        for st in range(2):
            x_sb = xpool.tile([P, D_IN], F32R, tag="x", name="x_sb")
            nc.sync.dma_start(out=x_sb, in_=x_d[st * P:(st + 1) * P, :])
            first_x.append(x_sb)

        for st in range(ST):
            if st < 2:
                x_sb = first_x[st]
            else:
                x_sb = xpool.tile([P, D_IN], F32R, tag="x", name="x_sb")
                nc.sync.dma_start(out=x_sb, in_=x_d[st * P:(st + 1) * P, :])

            xt_halves = []
            for h in range(2):
                ptx = ps_xt.tile([P, 4, P], F32R, tag=f"ptx{h}")
                for i in range(4):
                    kt = h * 4 + i
                    nc.tensor.transpose(
                        ptx[:, i], x_sb[:, kt * P:(kt + 1) * P], ident_r)
                xt_sb = xtpool.tile([P, 4, P], F32R, tag=f"xt{h}")
                nc.scalar.copy(out=xt_sb, in_=ptx)
                xt_halves.append(xt_sb)

            out_sb = opool.tile([P, D_OUT], F32, tag="o")
            pos = [ps_o.tile([P, 512], F32, tag=f"po{jh}", name=f"po{jh}") for jh in range(2)]
            for kt in range(KT):
                lhsT = xt_halves[kt // 4][:, kt % 4]
                for jh in range(2):
                    nc.tensor.matmul(
                        pos[jh], lhsT, w_tiles[kt][:, jh * 512:(jh + 1) * 512],
                        start=(kt == 0), stop=(kt == KT - 1))
            for jh in range(2):
                nc.vector.tensor_add(
                    out=out_sb[:, jh * 512:(jh + 1) * 512],
                    in0=pos[jh],
                    in1=bias_sb[:, jh * 512:(jh + 1) * 512])
            nc.scalar.dma_start(out=out_d[st * P:(st + 1) * P, :], in_=out_sb)

    nc.finalize()
    _NC_CACHE["nc"] = nc
    return nc


def _run_device(x, W_effT, bias_bcast, trace=False, tmpdir=None):
    """Run the SPMD kernel on 8 cores in this process. Returns (out, exec_ns)."""
    if _TRN_REPO not in sys.path:
        sys.path.insert(0, _TRN_REPO)
    from concourse.bass_utils import run_bass_kernel_spmd

    nc = _build_nc()
    core_ids = list(range(N_CORES))
    in_maps = [
        {"x": np.ascontiguousarray(x[c], dtype=np.float32),
         "wt": W_effT, "bias": bias_bcast}
        for c in core_ids
    ]
    res = run_bass_kernel_spmd(nc, in_maps, core_ids, trace=trace, tmpdir=tmpdir)
    out = np.stack([res.results[c]["out"] for c in core_ids], axis=0)
    return out, res.exec_time_ns


def _kernel_impl(x, in_perm, out_perm, weight_blocks, brow, bcol, bias, trace=False, tmpdir=None):
    x = np.asarray(x)
    W_effT, bias_bcast = _fold_weights(
        weight_blocks, brow, bcol, bias, in_perm, out_perm)
    out, exec_ns = _run_device(
        x.reshape(BATCH, SEQ, D_IN), W_effT, bias_bcast, trace=trace, tmpdir=tmpdir)
    return out.astype(np.float32), exec_ns


def kernel(x, in_perm, out_perm, weight_blocks, brow, bcol, bias):
    try:
        out, _ = _kernel_impl(
            x, in_perm, out_perm, weight_blocks, brow, bcol, bias)
        return out
    except Exception:
        # Fallback: run the device part in a clean subprocess (e.g. if the
        # calling process pinned JAX_PLATFORMS=cpu before importing jax).
        return _kernel_subprocess(
            x, in_perm, out_perm, weight_blocks, brow, bcol, bias)


def _kernel_subprocess(x, in_perm, out_perm, weight_blocks, brow, bcol, bias):
    with tempfile.TemporaryDirectory() as td:
        inp = os.path.join(td, "in.npz")
        outp = os.path.join(td, "out.npy")
        np.savez(inp, x=x, in_perm=in_perm, out_perm=out_perm,
                 weight_blocks=weight_blocks, brow=brow, bcol=bcol, bias=bias)
        env = dict(os.environ)
        env.pop("JAX_PLATFORMS", None)
        subprocess.run(
            [sys.executable, os.path.abspath(__file__), "--serve", inp, outp],
            check=True, env=env)
        return np.load(outp)


if __name__ == "__main__":
    if len(sys.argv) == 4 and sys.argv[1] == "--serve":
        data = np.load(sys.argv[2])
        out, _ = _kernel_impl(
            data["x"], data["in_perm"], data["out_perm"],
            data["weight_blocks"], data["brow"], data["bcol"], data["bias"])
        np.save(sys.argv[3], out)


# revision 7
# speedup vs baseline: 1.0164x; 1.0027x over previous
"""BlockSparseLinearWithPerm Trainium2 kernel.

Math: out[b,s,j] = sum_i x[b,s,in_perm[i]] * W[out_perm[j], i] + bias[out_perm[j]]
where W is the dense form of the block-sparse weight.

Both permutations and the block scatter are folded on the host into a dense
effective weight  W_effT[k, j] = sum_{i: in_perm[i]==k} W[out_perm[j], i]
(host cost: one 1024x1024 scatter-add — negligible), so the device kernel is a
pure dense matmul  out = x @ W_effT + bias_eff, data-parallel over the batch
dim: one batch element (8192x1024 tokens) per NeuronCore, weights replicated.

Device pipeline per 128-token tile (all matmul dtypes are float32r — full
1 cycle/row PE rate, tf32-class mantissa, ~1e-3 scale-relative error):
  DMA x tile [128, 1024] -> PE transpose k-tiles (via identity) -> PSUM
  -> ScalarE copy x^T to SBUF -> 16 accumulating f32r matmuls against the
  resident W_effT -> VectorE adds bias while copying PSUM -> SBUF -> DMA out.
"""
import os
import sys
import subprocess
import tempfile

import numpy as np

_TRN_REPO = "/opt/trn_rl_repo"

D_IN = 1024
D_OUT = 1024
BS = 64
R = D_OUT // BS
C = D_IN // BS
BATCH = 8
SEQ = 8192
P = 128
KT = D_IN // P          # 8 k-tiles
ST = SEQ // P           # 64 token tiles per core
N_CORES = 8


def _fold_weights(weight_blocks, brow, bcol, bias, in_perm, out_perm):
    """Fold block scatter + both permutations into W_effT [k, j] and bias_eff."""
    wb = np.asarray(weight_blocks, dtype=np.float64)
    brow = np.asarray(brow).astype(np.int64)
    bcol = np.asarray(bcol).astype(np.int64)
    in_perm = np.asarray(in_perm).astype(np.int64)
    out_perm = np.asarray(out_perm).astype(np.int64)
    W4 = np.zeros((R, C, BS, BS), dtype=np.float64)
    W4[brow, bcol] = wb
    W = W4.transpose(0, 2, 1, 3).reshape(D_OUT, D_IN)
    Wp = W[out_perm]                       # [j, i]
    W_effT = np.zeros((D_IN, D_OUT), dtype=np.float64)
    np.add.at(W_effT, in_perm, Wp.T)       # row i of Wp.T added into row in_perm[i]
    bias_eff = np.asarray(bias, dtype=np.float64)[out_perm]
    bias_bcast = np.broadcast_to(bias_eff, (P, D_OUT)).copy()
    return (np.ascontiguousarray(W_effT, dtype=np.float64).astype(np.float32),
            bias_bcast.astype(np.float32))


_NC_CACHE = {}


def _build_nc():
    if "nc" in _NC_CACHE:
        return _NC_CACHE["nc"]
    if _TRN_REPO not in sys.path:
        sys.path.insert(0, _TRN_REPO)
    import concourse.bacc as bacc
    import concourse.mybir as mybir
    from concourse.tile import TileContext
    from concourse.masks import make_identity
    from contextlib import ExitStack

    F32 = mybir.dt.float32
    F32R = mybir.dt.float32r

    nc = bacc.Bacc(target_bir_lowering=False)
    x_d = nc.declare_dram_parameter("x", [SEQ, D_IN], F32R, isOutput=False)
    wt_d = nc.declare_dram_parameter("wt", [D_IN, D_OUT], F32R, isOutput=False)
    bias_d = nc.declare_dram_parameter("bias", [P, D_OUT], F32, isOutput=False)
    out_d = nc.declare_dram_parameter("out", [SEQ, D_OUT], F32, isOutput=True)

    with TileContext(nc) as tc, ExitStack() as ctx:
        consts = ctx.enter_context(tc.tile_pool(name="consts", bufs=1))
        xpool = ctx.enter_context(tc.tile_pool(name="xpool", bufs=4))
        xtpool = ctx.enter_context(tc.tile_pool(name="xtpool", bufs=2))
        opool = ctx.enter_context(tc.tile_pool(name="opool", bufs=4))
        ps_xt = ctx.enter_context(tc.tile_pool(name="ps_xt", bufs=2, space="PSUM"))
        ps_o = ctx.enter_context(tc.tile_pool(name="ps_o", bufs=2, space="PSUM"))

        ident = consts.tile([P, P], F32)
        make_identity(nc, ident)
        ident_r = consts.tile([P, P], F32R)
        nc.vector.tensor_copy(out=ident_r, in_=ident)

        # x loads ride the SP HWDGE ring; W/bias ride the ACT HWDGE ring so the
        # 4.5MB of constants don't FIFO-block the first x tiles; out stores go
        # through SWDGE (gpsimd).  W is chunked per k-tile so the first matmuls
        # only wait on the first 512KB.
        wt_r = wt_d.rearrange("(kt p) j -> p kt j", p=P)
        bias_sb = consts.tile([P, D_OUT], F32)

        # W chunks 0-1 go FIRST on the sync ring so the first matmuls'
        # weights land before the x backlog; the rest stream on the scalar
        # ring in parallel with x loads.
        w_tiles = [consts.tile([P, D_OUT], F32R, name=f"w_{kt}")
                   for kt in range(KT)]
        first_x = []
        # interleave on the sync ring: x0 halves early, W0/W1 between them;
        # remaining W + bias on the scalar ring in parallel.
        x0 = xpool.tile([P, D_IN], F32R, tag="x", name="x_sb")
        nc.sync.dma_start(out=x0[:, :D_IN // 2], in_=x_d[0:P, :D_IN // 2])
        nc.sync.dma_start(out=w_tiles[0], in_=wt_r[:, 0])
        nc.sync.dma_start(out=x0[:, D_IN // 2:], in_=x_d[0:P, D_IN // 2:])
        nc.sync.dma_start(out=w_tiles[1], in_=wt_r[:, 1])
        first_x.append(x0)
        x1 = xpool.tile([P, D_IN], F32R, tag="x", name="x_sb")
        nc.sync.dma_start(out=x1, in_=x_d[P:2 * P, :])
        first_x.append(x1)
        for kt in range(2, KT):
            nc.scalar.dma_start(out=w_tiles[kt], in_=wt_r[:, kt])
        nc.scalar.dma_start(out=bias_sb, in_=bias_d[:, :])

        for st in range(ST):
            if st < 2:
                x_sb = first_x[st]
            else:
                x_sb = xpool.tile([P, D_IN], F32R, tag="x", name="x_sb")
                nc.sync.dma_start(out=x_sb, in_=x_d[st * P:(st + 1) * P, :])

            xt_halves = []
            for h in range(2):
                ptx = ps_xt.tile([P, 4, P], F32R, tag=f"ptx{h}")
                for i in range(4):
                    kt = h * 4 + i
                    nc.tensor.transpose(
                        ptx[:, i], x_sb[:, kt * P:(kt + 1) * P], ident_r)
                xt_sb = xtpool.tile([P, 4, P], F32R, tag=f"xt{h}")
                nc.scalar.copy(out=xt_sb, in_=ptx)
                xt_halves.append(xt_sb)

            out_sb = opool.tile([P, D_OUT], F32, tag="o")
            pos = [ps_o.tile([P, 512], F32, tag=f"po{jh}", name=f"po{jh}") for jh in range(2)]
            for jh in range(2):
                for kt in range(KT):
                    lhsT = xt_halves[kt // 4][:, kt % 4]
                    nc.tensor.matmul(
                        pos[jh], lhsT, w_tiles[kt][:, jh * 512:(jh + 1) * 512],
                        start=(kt == 0), stop=(kt == KT - 1))
            for jh in range(2):
                nc.vector.tensor_add(
                    out=out_sb[:, jh * 512:(jh + 1) * 512],
                    in0=pos[jh],
                    in1=bias_sb[:, jh * 512:(jh + 1) * 512])
            nc.scalar.dma_start(out=out_d[st * P:(st + 1) * P, :], in_=out_sb)

    nc.finalize()
    _NC_CACHE["nc"] = nc
    return nc


def _run_device(x, W_effT, bias_bcast, trace=False, tmpdir=None):
    """Run the SPMD kernel on 8 cores in this process. Returns (out, exec_ns)."""
    if _TRN_REPO not in sys.path:
        sys.path.insert(0, _TRN_REPO)
    from concourse.bass_utils import run_bass_kernel_spmd

    nc = _build_nc()
    core_ids = list(range(N_CORES))
    in_maps = [
        {"x": np.ascontiguousarray(x[c], dtype=np.float32),
         "wt": W_effT, "bias": bias_bcast}
        for c in core_ids
    ]
    res = run_bass_kernel_spmd(nc, in_maps, core_ids, trace=trace, tmpdir=tmpdir)
    out = np.stack([res.results[c]["out"] for c in core_ids], axis=0)
    return out, res.exec_time_ns


def _kernel_impl(x, in_perm, out_perm, weight_blocks, brow, bcol, bias, trace=False, tmpdir=None):
    x = np.asarray(x)
    W_effT, bias_bcast = _fold_weights(
        weight_blocks, brow, bcol, bias, in_perm, out_perm)
    out, exec_ns = _run_device(
        x.reshape(BATCH, SEQ, D_IN), W_effT, bias_bcast, trace=trace, tmpdir=tmpdir)
    return out.astype(np.float32), exec_ns


def kernel(x, in_perm, out_perm, weight_blocks, brow, bcol, bias):
    try:
        out, _ = _kernel_impl(
            x, in_perm, out_perm, weight_blocks, brow, bcol, bias)
        return out
    except Exception:
        # Fallback: run the device part in a clean subprocess (e.g. if the
        # calling process pinned JAX_PLATFORMS=cpu before importing jax).
        return _kernel_subprocess(
            x, in_perm, out_perm, weight_blocks, brow, bcol, bias)


def _kernel_subprocess(x, in_perm, out_perm, weight_blocks, brow, bcol, bias):
    with tempfile.TemporaryDirectory() as td:
        inp = os.path.join(td, "in.npz")
        outp = os.path.join(td, "out.npy")
        np.savez(inp, x=x, in_perm=in_perm, out_perm=out_perm,
                 weight_blocks=weight_blocks, brow=brow, bcol=bcol, bias=bias)
        env = dict(os.environ)
        env.pop("JAX_PLATFORMS", None)
        subprocess.run(
            [sys.executable, os.path.abspath(__file__), "--serve", inp, outp],
            check=True, env=env)
        return np.load(outp)


if __name__ == "__main__":
    if len(sys.argv) == 4 and sys.argv[1] == "--serve":
        data = np.load(sys.argv[2])
        out, _ = _kernel_impl(
            data["x"], data["in_perm"], data["out_perm"],
            data["weight_blocks"], data["brow"], data["bcol"], data["bias"])
        np.save(sys.argv[3], out)


# revision 8
# speedup vs baseline: 1.2105x; 1.1910x over previous
"""BlockSparseLinearWithPerm Trainium2 kernel.

Math: out[b,s,j] = sum_i x[b,s,in_perm[i]] * W[out_perm[j], i] + bias[out_perm[j]]
where W is the dense form of the block-sparse weight.

Both permutations and the block scatter are folded on the host into a dense
effective weight  W_effT[k, j] = sum_{i: in_perm[i]==k} W[out_perm[j], i]
(host cost: one 1024x1024 scatter-add — negligible), so the device kernel is a
pure dense matmul  out = x @ W_effT + bias_eff, data-parallel over the batch
dim: one batch element (8192x1024 tokens) per NeuronCore, weights replicated.

Sharding/layout: each core's x slice is shipped feature-major (x^T) so the
contraction dim lands on SBUF partitions directly — the device spends zero
TensorE cycles on transposes and runs a pure f32r matmul stream (full
1 cycle/row PE rate, tf32-class mantissa, ~1e-3 scale-relative error).
Per 128-token tile: 16 accumulating f32r matmuls (lhsT = x^T k-tiles,
moving = resident W_effT) -> VectorE adds bias while copying PSUM -> SBUF
-> DMA out in natural token-major layout.
"""
import os
import sys
import subprocess
import tempfile

import numpy as np

_TRN_REPO = "/opt/trn_rl_repo"

D_IN = 1024
D_OUT = 1024
BS = 64
R = D_OUT // BS
C = D_IN // BS
BATCH = 8
SEQ = 8192
P = 128
KT = D_IN // P          # 8 k-tiles
WIN = 1024              # tokens per x^T window
NWIN = SEQ // WIN       # 8 windows
N_CORES = 8


def _fold_weights(weight_blocks, brow, bcol, bias, in_perm, out_perm):
    """Fold block scatter + both permutations into W_effT [k, j] and bias_eff."""
    wb = np.asarray(weight_blocks, dtype=np.float64)
    brow = np.asarray(brow).astype(np.int64)
    bcol = np.asarray(bcol).astype(np.int64)
    in_perm = np.asarray(in_perm).astype(np.int64)
    out_perm = np.asarray(out_perm).astype(np.int64)
    W4 = np.zeros((R, C, BS, BS), dtype=np.float64)
    W4[brow, bcol] = wb
    W = W4.transpose(0, 2, 1, 3).reshape(D_OUT, D_IN)
    Wp = W[out_perm]                       # [j, i]
    W_effT = np.zeros((D_IN, D_OUT), dtype=np.float64)
    np.add.at(W_effT, in_perm, Wp.T)       # row i of Wp.T added into row in_perm[i]
    bias_eff = np.asarray(bias, dtype=np.float64)[out_perm]
    bias_bcast = np.broadcast_to(bias_eff, (P, D_OUT)).copy()
    return (np.ascontiguousarray(W_effT, dtype=np.float64).astype(np.float32),
            bias_bcast.astype(np.float32))


_NC_CACHE = {}


def _build_nc():
    if "nc" in _NC_CACHE:
        return _NC_CACHE["nc"]
    if _TRN_REPO not in sys.path:
        sys.path.insert(0, _TRN_REPO)
    import concourse.bacc as bacc
    import concourse.mybir as mybir
    from concourse.tile import TileContext
    from contextlib import ExitStack

    F32 = mybir.dt.float32
    F32R = mybir.dt.float32r

    nc = bacc.Bacc(target_bir_lowering=False)
    xt_d = nc.declare_dram_parameter("xt", [D_IN, SEQ], F32R, isOutput=False)
    wt_d = nc.declare_dram_parameter("wt", [D_IN, D_OUT], F32R, isOutput=False)
    bias_d = nc.declare_dram_parameter("bias", [P, D_OUT], F32, isOutput=False)
    out_d = nc.declare_dram_parameter("out", [SEQ, D_OUT], F32, isOutput=True)

    xt_r = xt_d.rearrange("(kt p) s -> p kt s", p=P)
    wt_r = wt_d.rearrange("(kt p) j -> p kt j", p=P)

    with TileContext(nc) as tc, ExitStack() as ctx:
        consts = ctx.enter_context(tc.tile_pool(name="consts", bufs=1))
        xpool = ctx.enter_context(tc.tile_pool(name="xpool", bufs=2))
        opool = ctx.enter_context(tc.tile_pool(name="opool", bufs=4))
        ps_o = ctx.enter_context(tc.tile_pool(name="ps_o", bufs=4, space="PSUM"))

        bias_sb = consts.tile([P, D_OUT], F32)
        # First x^T k-chunk goes FIRST on the sync ring so the first matmul
        # chain unblocks early; W streams on the scalar ring in parallel.
        w_tiles = [consts.tile([P, D_OUT], F32R, name=f"w_{kt}")
                   for kt in range(KT)]
        xwin0 = xpool.tile([P, KT, WIN], F32R, tag="xw", name="xwin")
        nc.sync.dma_start(out=xwin0[:, 0], in_=xt_r[:, 0, 0:WIN])
        nc.scalar.dma_start(out=w_tiles[0], in_=wt_r[:, 0])
        for kt in range(1, KT):
            nc.sync.dma_start(out=xwin0[:, kt], in_=xt_r[:, kt, 0:WIN])
            nc.scalar.dma_start(out=w_tiles[kt], in_=wt_r[:, kt])
        nc.scalar.dma_start(out=bias_sb, in_=bias_d[:, :])

        for win in range(NWIN):
            if win == 0:
                xwin = xwin0
            else:
                xwin = xpool.tile([P, KT, WIN], F32R, tag="xw", name="xwin")
                nc.sync.dma_start(
                    out=xwin, in_=xt_r[:, :, win * WIN:(win + 1) * WIN])
            for ss in range(WIN // P):
                s_lo = ss * P
                out_sb = opool.tile([P, D_OUT], F32, tag="o", name="out_sb")
                pos = [ps_o.tile([P, 512], F32, tag=f"po{jh}", name=f"po{jh}")
                       for jh in range(2)]
                for jh in range(2):
                    for kt in range(KT):
                        nc.tensor.matmul(
                            pos[jh],
                            xwin[:, kt, s_lo:s_lo + P],
                            w_tiles[kt][:, jh * 512:(jh + 1) * 512],
                            start=(kt == 0), stop=(kt == KT - 1))
                for jh in range(2):
                    nc.vector.tensor_add(
                        out=out_sb[:, jh * 512:(jh + 1) * 512],
                        in0=pos[jh],
                        in1=bias_sb[:, jh * 512:(jh + 1) * 512])
                st = win * (WIN // P) + ss
                nc.scalar.dma_start(
                    out=out_d[st * P:(st + 1) * P, :], in_=out_sb)

    nc.finalize()
    _NC_CACHE["nc"] = nc
    return nc


def _run_device(x, W_effT, bias_bcast, trace=False, tmpdir=None):
    """Run the SPMD kernel on 8 cores in this process. Returns (out, exec_ns)."""
    if _TRN_REPO not in sys.path:
        sys.path.insert(0, _TRN_REPO)
    from concourse.bass_utils import run_bass_kernel_spmd

    nc = _build_nc()
    core_ids = list(range(N_CORES))
    in_maps = [
        {"xt": np.ascontiguousarray(np.asarray(x[c], dtype=np.float32).T),
         "wt": W_effT, "bias": bias_bcast}
        for c in core_ids
    ]
    res = run_bass_kernel_spmd(nc, in_maps, core_ids, trace=trace, tmpdir=tmpdir)
    out = np.stack([res.results[c]["out"] for c in core_ids], axis=0)
    return out, res.exec_time_ns


def _kernel_impl(x, in_perm, out_perm, weight_blocks, brow, bcol, bias,
                 trace=False, tmpdir=None):
    x = np.asarray(x)
    W_effT, bias_bcast = _fold_weights(
        weight_blocks, brow, bcol, bias, in_perm, out_perm)
    out, exec_ns = _run_device(
        x.reshape(BATCH, SEQ, D_IN), W_effT, bias_bcast,
        trace=trace, tmpdir=tmpdir)
    return out.astype(np.float32), exec_ns


def kernel(x, in_perm, out_perm, weight_blocks, brow, bcol, bias):
    try:
        out, _ = _kernel_impl(
            x, in_perm, out_perm, weight_blocks, brow, bcol, bias)
        return out
    except Exception:
        # Fallback: run the device part in a clean subprocess (e.g. if the
        # calling process pinned JAX_PLATFORMS=cpu before importing jax).
        return _kernel_subprocess(
            x, in_perm, out_perm, weight_blocks, brow, bcol, bias)


def _kernel_subprocess(x, in_perm, out_perm, weight_blocks, brow, bcol, bias):
    with tempfile.TemporaryDirectory() as td:
        inp = os.path.join(td, "in.npz")
        outp = os.path.join(td, "out.npy")
        np.savez(inp, x=x, in_perm=in_perm, out_perm=out_perm,
                 weight_blocks=weight_blocks, brow=brow, bcol=bcol, bias=bias)
        env = dict(os.environ)
        env.pop("JAX_PLATFORMS", None)
        subprocess.run(
            [sys.executable, os.path.abspath(__file__), "--serve", inp, outp],
            check=True, env=env)
        return np.load(outp)


if __name__ == "__main__":
    if len(sys.argv) == 4 and sys.argv[1] == "--serve":
        data = np.load(sys.argv[2])
        out, _ = _kernel_impl(
            data["x"], data["in_perm"], data["out_perm"],
            data["weight_blocks"], data["brow"], data["bcol"], data["bias"])
        np.save(sys.argv[3], out)
